# revision 14
# baseline (speedup 1.0000x reference)
"""3-layer GAT on Trainium2, 8-core SPMD Bass kernel (v2).

Strategy (dst-partitioned, edge-gather based):
  - Nodes partitioned contiguously across 8 cores (6250/core). Each core owns
    all edges whose dst lands in its range; segment-softmax and the weighted
    scatter-sum are core-local.
  - Per layer: per 128-node block, z_aug = x @ [W.T | W.T a_src | W.T a_dst]
    computed on PE, fp16 table rows [z | 1 | 0 | s(f32) | t(f32)] written to
    DRAM, AllGather replicates the node table to every core.
  - Edge phase, per 128-dst block: one dma_gather per (block, src-half) pulls
    z_aug rows of the block's edge sources (trailing -1 indices skip padding
    descriptors; per-core valid counts come from an SBUF table via
    value_load). Attention: e = s[src] + t[dst] (t via one-hot mask reduce),
    leaky_relu, exp (shift-free: f32 exp exact for softmax). Per-dst sums via
    PE matmul U = PT_ex.T @ Zg with a constant-1 column giving the
    denominator. Scalar engine handles exp/reciprocal/normalize/tanh.
  - The next layer's z-phase for block b is emitted right after block b's
    edge compute so layers overlap; AllGather is the only barrier.
"""

import os
import sys

import numpy as np

sys.path.insert(0, "/opt/trn_rl_repo")

import ml_dtypes  # noqa: E402

PAD_FULL = bool(int(os.environ.get("GAT_PAD_FULL", "1")))

# --- problem constants (hardcoded per contest rules) ---
N_NODES = 50000
N_EDGES = 800000
DIM_IN = 256
DIM_HID = 256
DIM_OUT = 128
N_CORES = 8

BLOCK = 128  # dst nodes per block (one U psum, seg ids 0..127)
MAXCK = 8    # max chunks (x128 descriptors) per dma_gather call (ring capacity)
P = 128

NEG_SLOPE = 0.01


def _cdiv(a, b):
    return -(-a // b)


# ---------------------------------------------------------------------------
# host-side graph plan (pure index preprocessing)
# ---------------------------------------------------------------------------

def build_plan(src, dst, n_nodes, n_cores):
    src = np.asarray(src).astype(np.int64)
    dst = np.asarray(dst).astype(np.int64)
    assert n_nodes % n_cores == 0
    npc = n_nodes // n_cores
    H = (n_nodes + 1) // 2
    assert max(H, n_nodes - H) <= 32767

    n_blocks = _cdiv(npc, BLOCK)
    c = dst // npc
    loc = dst - c * npc
    b = loc // BLOCK
    sg = loc % BLOCK
    hf = (src >= H).astype(np.int64)

    key = (c * n_blocks + b) * 2 + hf
    cnt = np.bincount(key, minlength=n_cores * n_blocks * 2).reshape(
        n_cores, n_blocks, 2
    )
    mx = cnt.max(axis=0)  # [n_blocks, 2]
    ck = (mx + P - 1) // P  # chunks per (block, half), shared across cores
    dead = ck.sum(axis=1) == 0
    ck[dead, 0] = 1  # keep >=1 chunk per block so PSUM init happens

    order = np.lexsort((src, hf, b, c))
    gsize = cnt.reshape(-1)
    gstart = np.zeros_like(gsize)
    gstart[1:] = np.cumsum(gsize)[:-1]

    # block/call layout (shared across cores)
    blocks = []
    seg_cols = lo_cols = hi_cols = 0
    ncalls = 0
    for bb in range(n_blocks):
        cklo, ckhi = int(ck[bb, 0]), int(ck[bb, 1])
        calls = []
        for h, ckh, off8 in ((0, cklo, lo_cols), (1, ckhi, hi_cols)):
            dbase = 0 if h == 0 else cklo
            for c0 in range(0, ckh, MAXCK):
                nck = min(MAXCK, ckh - c0)
                calls.append(
                    dict(h=h, c0=dbase + c0, u0=c0, nck=nck,
                         ci=ncalls, off8=off8 + c0 * 8)
                )
                ncalls += 1
        blocks.append(
            dict(cs=seg_cols, cklo=cklo, ckhi=ckhi, ctot=cklo + ckhi,
                 lo8=lo_cols, hi8=hi_cols, calls=calls)
        )
        seg_cols += cklo + ckhi
        lo_cols += cklo * 8
        hi_cols += ckhi * 8

    ncalls_pad = _cdiv(ncalls, 16) * 16

    def wrap16(a):
        # idx j -> partition j%16, col j//16; replicated to 8 groups of 16
        S = len(a) // 16
        w = a.reshape(S, 16).T
        return np.tile(w, (8, 1))

    idxlo = np.full((n_cores, P, lo_cols), -1, np.int16)
    idxhi = np.full((n_cores, P, hi_cols), -1, np.int16)
    seg = np.full((n_cores, P, seg_cols), 255.0, np.float32)
    cnts = np.zeros((n_cores, 1, ncalls_pad), np.int32)

    for cc_ in range(n_cores):
        for bb, blk in enumerate(blocks):
            ctot = blk["ctot"]
            seg_slots = np.full(ctot * P, 255, np.int64)
            for h, ckh, off8, idxarr, sbase in (
                (0, blk["cklo"], blk["lo8"], idxlo, 0),
                (1, blk["ckhi"], blk["hi8"], idxhi, blk["cklo"] * P),
            ):
                if ckh == 0:
                    continue
                n = cnt[cc_, bb, h]
                s0 = gstart[(cc_ * n_blocks + bb) * 2 + h]
                e = order[s0 : s0 + n]
                vals = np.full(ckh * P, -1, np.int64)
                vals[:n] = src[e] - (H if h else 0)
                seg_slots[sbase : sbase + n] = sg[e]
                # per-call dummy fill + counts
                for call in blk["calls"]:
                    if call["h"] != h:
                        continue
                    lo_s = call["u0"] * P
                    cap = call["nck"] * P
                    v = min(max(n - lo_s, 0), cap)
                    if v == 0:
                        vals[lo_s] = 0  # dummy valid idx (seg stays 255)
                        v = 1
                    # the SWDGE ring booking must match the descriptors the
                    # ucode actually pushes: valid-prefix count with trailing
                    # -1 trim, full capacity when padded with dummy indices
                    cnts[cc_, 0, call["ci"]] = cap if PAD_FULL else v
                if PAD_FULL:
                    vals[vals < 0] = 0
                idxarr[cc_, :, off8 : off8 + ckh * 8] = wrap16(vals)
            seg[cc_, :, blk["cs"] : blk["cs"] + ctot] = seg_slots.reshape(
                ctot, P
            ).T

    meta = dict(
        n_cores=n_cores,
        n_nodes=n_nodes,
        npc=npc,
        H=H,
        n_blocks=n_blocks,
        blocks=blocks,
        lo_cols=lo_cols,
        hi_cols=hi_cols,
        seg_cols=seg_cols,
        ncalls=ncalls,
        ncalls_pad=ncalls_pad,
        ctot_max=max(bl["ctot"] for bl in blocks),
    )
    per_core = dict(
        idxlo=idxlo,
        idxhi=idxhi,
        seg=seg.astype(ml_dtypes.bfloat16),
        cnts=cnts,
    )
    return meta, per_core


def const_inputs():
    iota = np.tile(np.arange(BLOCK, dtype=np.float32), (P, 1)).astype(
        ml_dtypes.bfloat16
    )
    id16 = np.eye(P, dtype=np.float16)
    id32 = np.eye(P, dtype=np.float32)
    return {"iota": iota, "id16": id16, "id32": id32}


def build_waug(W, A):
    d_out = W.shape[0]
    Wt = W.T.astype(np.float64)
    a_s = A[0, :d_out].astype(np.float64)
    a_d = A[0, d_out:].astype(np.float64)
    waug = np.concatenate([Wt, (Wt @ a_s)[:, None], (Wt @ a_d)[:, None]], axis=1)
    return waug.astype(np.float16)


# ---------------------------------------------------------------------------
# device program
# ---------------------------------------------------------------------------

def build_nc(meta, dims, debug=False, timing_single_core=False,
             no_collective=False):
    import concourse.bacc as bacc
    import concourse.bass as bass
    import concourse.mybir as mybir
    import concourse.tile as tile
    from concourse.library_config import mlp

    dt = mybir.dt
    AP = bass.AP
    ALU = bass.mybir.AluOpType
    ACT = bass.mybir.ActivationFunctionType
    d_in, d_hid, d_out = dims
    npc = meta["npc"]
    H = meta["H"]
    N = meta["n_nodes"]
    n_blocks = meta["n_blocks"]
    n_cores = meta["n_cores"]
    blocks = meta["blocks"]
    ctmax = meta["ctot_max"]
    KCH = d_in // P  # contraction chunks (2)
    assert d_in == d_hid == 2 * P and d_out == P

    #        (Din,  Dout,  act,    stride)
    LYR = [
        (d_in, d_hid, "tanh", 384),
        (d_hid, d_hid, "elu", 384),
        (d_hid, d_out, None, 256),
    ]

    nc = bacc.Bacc(
        "TRN2", target_bir_lowering=False, debug=debug,
        num_devices=1 if timing_single_core else n_cores,
    )

    h_in = nc.dram_tensor("h", [npc, d_in], dt.float32, kind="ExternalInput")
    w_in = [
        nc.dram_tensor(f"w{l}", [LYR[l][0], LYR[l][1] + 2], dt.float16,
                       kind="ExternalInput")
        for l in range(3)
    ]
    ixlo_in = nc.dram_tensor("idxlo", [P, meta["lo_cols"]], dt.int16,
                             kind="ExternalInput")
    ixhi_in = nc.dram_tensor("idxhi", [P, meta["hi_cols"]], dt.int16,
                             kind="ExternalInput")
    seg_in = nc.dram_tensor("seg", [P, meta["seg_cols"]], dt.bfloat16,
                            kind="ExternalInput")
    cnts_in = nc.dram_tensor("cnts", [1, meta["ncalls_pad"]], dt.int32,
                             kind="ExternalInput")
    iota_in = nc.dram_tensor("iota", [P, BLOCK], dt.bfloat16, kind="ExternalInput")
    id16_in = nc.dram_tensor("id16", [P, P], dt.float16, kind="ExternalInput")
    id32_in = nc.dram_tensor("id32", [P, P], dt.float32, kind="ExternalInput")
    out_t = nc.dram_tensor("out", [npc, d_out], dt.float32, kind="ExternalOutput")

    agi = [nc.dram_tensor(f"agi{l}", [npc, LYR[l][3]], dt.float16) for l in range(3)]
    ago = [
        nc.dram_tensor(f"ago{l}", [npc * n_cores, LYR[l][3]], dt.float16,
                       addr_space="Shared")
        for l in range(3)
    ]

    def bc_mid(ap2, n):
        # [P, W] -> [P, n, W] broadcasting a middle dim
        return AP(ap2.tensor, ap2.offset, [ap2.ap[0], [0, n], ap2.ap[1]])

    def bc_last(ap2, n):
        # [P, W] -> [P, W, n] broadcasting the last dim
        return AP(ap2.tensor, ap2.offset, [ap2.ap[0], ap2.ap[1], [0, n]])

    with tile.TileContext(nc) as tc:
        import contextlib

        ctx = contextlib.ExitStack()
        with ctx:
            pers = ctx.enter_context(tc.tile_pool(name="pers", bufs=1))
            pg = ctx.enter_context(tc.tile_pool(name="pg", bufs=3))
            ppt = ctx.enter_context(tc.tile_pool(name="ppt", bufs=2))
            psm = ctx.enter_context(tc.tile_pool(name="psm", bufs=4))
            pz = ctx.enter_context(tc.tile_pool(name="pz", bufs=3))
            px = ctx.enter_context(tc.tile_pool(name="px", bufs=2))
            psum_z = ctx.enter_context(tc.tile_pool(name="psz", bufs=2, space="PSUM"))
            psum_u = ctx.enter_context(tc.tile_pool(name="psu", bufs=2, space="PSUM"))
            psum_tr = ctx.enter_context(tc.tile_pool(name="pstr", bufs=2, space="PSUM"))
            psum_tt = ctx.enter_context(tc.tile_pool(name="pstt", bufs=2, space="PSUM"))

            nc.gpsimd.load_library(mlp)

            # persistent state
            seg_sb = pers.tile([P, meta["seg_cols"]], dt.bfloat16, tag="seg", name="seg_sb")
            ixlo_sb = pers.tile([P, meta["lo_cols"]], dt.int16, tag="ixlo", name="ixlo_sb")
            ixhi_sb = pers.tile([P, meta["hi_cols"]], dt.int16, tag="ixhi", name="ixhi_sb")
            cnt_sb = pers.tile([1, meta["ncalls_pad"]], dt.int32, tag="cnts", name="cnt_sb")
            W_sb = [pers.tile([P, KCH, LYR[l][1] + 2], dt.float16, tag=f"w{l}", name=f"wsb{l}")
                    for l in range(3)]
            iota_sb = pers.tile([P, BLOCK], dt.bfloat16, tag="iota", name="iota_sb")
            id16 = pers.tile([P, P], dt.float16, tag="id16", name="id16")
            id32 = pers.tile([P, P], dt.float32, tag="id32", name="id32")
            t_row = [
                pers.tile([1, n_blocks * BLOCK], dt.bfloat16, tag=f"trow{par}",
                          name=f"t_row{par}")
                for par in range(2)
            ]
            xT = [
                [
                    [pers.tile([P, BLOCK], dt.float16, tag=f"xT{par}_{k}_{bb}",
                               name=f"xT{par}_{k}_{bb}") for bb in range(n_blocks)]
                    for k in range(KCH)
                ]
                for par in range(2)
            ]

            nc.sync.dma_start(out=seg_sb[:], in_=seg_in[:, :])
            nc.sync.dma_start(out=ixlo_sb[:], in_=ixlo_in[:, :])
            nc.sync.dma_start(out=ixhi_sb[:], in_=ixhi_in[:, :])
            nc.sync.dma_start(out=cnt_sb[:], in_=cnts_in[:, :])
            for l in range(3):
                nc.sync.dma_start(
                    out=W_sb[l][:],
                    in_=w_in[l].ap().rearrange("(k p) d -> p k d", p=P),
                )
            nc.sync.dma_start(out=iota_sb[:], in_=iota_in[:, :])
            nc.sync.dma_start(out=id16[:], in_=id16_in[:, :])
            nc.sync.dma_start(out=id32[:], in_=id32_in[:, :])
            nc.vector.memset(t_row[0][:], 0.0)
            nc.vector.memset(t_row[1][:], 0.0)

            def z_phase_block(l, bb, bn):
                """z_aug for nodes of block bb of layer l -> agi[l] rows."""
                Dout = LYR[l][1]
                DU = Dout + 2
                ASM = Dout + 6
                par = l % 2
                zp = psum_z.tile([P, 258], dt.float32, tag="zp", name="zp")
                for k in range(KCH):
                    nc.tensor.matmul(
                        out=zp[:bn, :DU],
                        lhsT=xT[par][k][bb][:, :bn],
                        rhs=W_sb[l][:, k, :DU],
                        start=(k == 0),
                        stop=(k == KCH - 1),
                    )
                asm = pz.tile([P, 262], dt.float16, tag="asm", name="asm")
                nc.scalar.activation(asm[:bn, 0:Dout], zp[:bn, 0:Dout], ACT.Copy)
                nc.vector.memset(asm[:bn, Dout : Dout + 1], 1.0)
                nc.vector.memset(asm[:bn, Dout + 1 : Dout + 2], 0.0)
                nc.vector.tensor_copy(
                    out=asm[:bn, Dout + 2 : Dout + 6].bitcast(dt.float32),
                    in_=zp[:bn, Dout : Dout + 2],
                )
                tcol = pz.tile([P, 1], dt.float32, tag="tcol", name="tcol")
                nc.vector.tensor_copy(out=tcol[:bn], in_=zp[:bn, Dout + 1 : Dout + 2])
                tp = psum_tt.tile([1, P], dt.float32)
                nc.tensor.transpose(
                    out=tp[:1, :bn], in_=tcol[:bn, :], identity=id32[:bn, :bn],
                )
                nc.vector.tensor_copy(
                    out=t_row[par][0:1, bb * BLOCK : bb * BLOCK + bn],
                    in_=tp[0:1, :bn],
                )
                nc.sync.dma_start(
                    out=agi[l][bb * BLOCK : bb * BLOCK + bn, 0:ASM],
                    in_=asm[:bn, 0:ASM],
                )

            def allgather(l):
                if timing_single_core or no_collective:
                    nc.sync.dma_start(out=ago[l].ap()[0:npc, :], in_=agi[l].ap()[:, :])
                else:
                    nc.gpsimd.collective_compute(
                        "AllGather",
                        ALU.bypass,
                        replica_groups=[list(range(n_cores))],
                        ins=[agi[l].ap().opt()],
                        outs=[ago[l].ap().opt()],
                    )

            # ---- layer-0 input: load h, cast fp16, transpose to xT[0] ----
            for bb in range(n_blocks):
                bn = min(BLOCK, npc - bb * BLOCK)
                ht = pz.tile([P, d_in], dt.float32, tag="ht", name="ht")
                nc.sync.dma_start(out=ht[:bn], in_=h_in[bb * BLOCK : bb * BLOCK + bn, :])
                h16 = pz.tile([P, d_in], dt.float16, tag="h16", name="h16")
                nc.scalar.activation(h16[:bn], ht[:bn], ACT.Copy)
                for k in range(KCH):
                    ps = psum_tr.tile([P, P], dt.float16)
                    nc.tensor.transpose(
                        out=ps[:P, :bn],
                        in_=h16[:bn, k * P : (k + 1) * P],
                        identity=id16[:bn, :bn],
                    )
                    nc.vector.tensor_copy(out=xT[0][k][bb][:, :bn], in_=ps[:, :bn])
                z_phase_block(0, bb, bn)
            allgather(0)

            # ---- layers ----
            qn = [0]  # gather call counter
            cnt_regs = [nc.gpsimd.alloc_register(f"cntreg{i}") for i in range(4)]

            for l in range(3):
                Din, Dout, act, STRIDE = LYR[l]
                ELEM = STRIDE
                DU = Dout + 2
                SOFF = Dout + 2
                par = l % 2
                last = l == 2

                lo_tab = ago[l].ap()[0:H, 0:ELEM]
                hi_tab = ago[l].ap()[H:N, 0:ELEM]

                # re-zero the g ring buffers: stale bytes from the previous
                # layer's layout could bitcast to inf/nan in pad slots
                for _ in range(3):
                    gg = pg.tile([P, ctmax, ELEM], dt.float16, tag="g", name="g")
                    nc.vector.memset(gg[:], 0.0)

                for bb, blk in enumerate(blocks):
                    bn = min(BLOCK, npc - bb * BLOCK)
                    cs = blk["cs"]
                    ctot = blk["ctot"]
                    g = pg.tile([P, ctmax, ELEM], dt.float16, tag="g", name="g")
                    for call in blk["calls"]:
                        tabsrc = hi_tab if call["h"] else lo_tab
                        ixsb = ixhi_sb if call["h"] else ixlo_sb
                        nck = call["nck"]
                        # num_idxs_reg must be the per-core valid count: the
                        # ucode trims trailing -1 indices and the SWDGE ring
                        # booking must match the descriptors actually pushed
                        reg = cnt_regs[qn[0] % 4]
                        nc.gpsimd.reg_load(
                            reg, cnt_sb[0:1, call["ci"] : call["ci"] + 1]
                        )
                        nc.gpsimd.dma_gather(
                            g[:, call["c0"] : call["c0"] + nck, :],
                            tabsrc,
                            ixsb[:, call["off8"] : call["off8"] + nck * 8],
                            nck * P,
                            reg,
                            ELEM,
                            elem_step=STRIDE,
                        )
                        qn[0] += 1

                    # t broadcast for this block's dst nodes
                    tb = psm.tile([P, BLOCK], dt.bfloat16, tag="tb", name="tb")
                    nc.gpsimd.partition_broadcast(
                        tb[:],
                        t_row[par][0:1, bb * BLOCK : bb * BLOCK + BLOCK],
                        channels=P,
                    )
                    seg_v = seg_sb[:, cs : cs + ctot]
                    pt0 = ppt.tile([P, ctmax, BLOCK], dt.bfloat16, tag="pt0", name="pt0")
                    nc.vector.tensor_tensor(
                        out=pt0[:, :ctot, :],
                        in0=bc_last(seg_v, BLOCK),
                        in1=bc_mid(iota_sb[:], ctot),
                        op=ALU.is_equal,
                    )
                    ptm = ppt.tile([P, ctmax, BLOCK], dt.bfloat16, tag="ptm", name="ptm")
                    nc.vector.tensor_tensor(
                        out=ptm[:, :ctot, :],
                        in0=pt0[:, :ctot, :],
                        in1=bc_mid(tb[:], ctot),
                        op=ALU.mult,
                    )
                    tsel = psm.tile([P, ctmax], dt.float32, tag="tsel", name="tsel")
                    nc.vector.tensor_reduce(
                        out=tsel[:, :ctot],
                        in_=ptm[:, :ctot, :],
                        axis=bass.mybir.AxisListType.X,
                        op=ALU.add,
                    )
                    # e = s + t  (s: f32 bits at fp16 cols SOFF..SOFF+2)
                    sv = g[:, 0:ctot, SOFF : SOFF + 2].bitcast(dt.float32)
                    sv2 = AP(sv.tensor, sv.offset, [sv.ap[0], sv.ap[1]])
                    e0 = psm.tile([P, ctmax], dt.float32, tag="e0", name="e0")
                    nc.vector.tensor_tensor(
                        out=e0[:, :ctot], in0=sv2, in1=tsel[:, :ctot], op=ALU.add,
                    )
                    e1 = psm.tile([P, ctmax], dt.float32, tag="e1", name="e1")
                    nc.vector.tensor_scalar(
                        out=e1[:, :ctot], in0=e0[:, :ctot], scalar1=NEG_SLOPE,
                        scalar2=None, op0=ALU.mult,
                    )
                    e2 = psm.tile([P, ctmax], dt.float32, tag="e2", name="e2")
                    nc.vector.tensor_tensor(
                        out=e2[:, :ctot], in0=e0[:, :ctot], in1=e1[:, :ctot],
                        op=ALU.max,
                    )
                    exb = psm.tile([P, ctmax], dt.bfloat16, tag="exb", name="exb")
                    nc.scalar.activation(exb[:, :ctot], e2[:, :ctot], ACT.Exp)
                    ptx = ppt.tile([P, ctmax, BLOCK], dt.bfloat16, tag="ptx", name="ptx")
                    nc.vector.tensor_tensor(
                        out=ptx[:, :ctot, :],
                        in0=pt0[:, :ctot, :],
                        in1=bc_last(exb[:, :ctot], BLOCK),
                        op=ALU.mult,
                    )
                    U = psum_u.tile([P, 258], dt.float32, tag="U", name="U")
                    for k in range(ctot):
                        nc.tensor.matmul(
                            out=U[:, :DU],
                            lhsT=ptx[:, k, :],
                            rhs=g[:, k, 0:DU],
                            start=(k == 0),
                            stop=(k == ctot - 1),
                        )
                    den = psm.tile([P, 1], dt.float32, tag="den", name="den")
                    nc.vector.tensor_scalar(
                        out=den[:bn],
                        in0=U[:bn, Dout : Dout + 1],
                        scalar1=1e-9, scalar2=None, op0=ALU.max,
                    )
                    rec = psm.tile([P, 1], dt.float32, tag="rec", name="rec")
                    nc.vector.reciprocal(rec[:bn], den[:bn])
                    if last:
                        ox = px.tile([P, d_out], dt.float32, tag="ox", name="ox")
                        nc.scalar.activation(
                            ox[:bn], U[:bn, 0:d_out], ACT.Copy, scale=rec[:bn],
                        )
                        nc.sync.dma_start(
                            out=out_t[bb * BLOCK : bb * BLOCK + bn, :],
                            in_=ox[:bn, :],
                        )
                        continue
                    # activation + transpose into next layer's xT
                    a16 = px.tile([P, 256], dt.float16, tag="a16", name="a16")
                    if act == "tanh":
                        nc.scalar.activation(
                            a16[:bn], U[:bn, 0:Dout], ACT.Tanh, scale=rec[:bn],
                        )
                    else:  # elu = max(x,0) + exp(min(x,0)) - 1
                        xv = px.tile([P, 256], dt.float32, tag="xv", name="xv")
                        nc.scalar.activation(
                            xv[:bn], U[:bn, 0:Dout], ACT.Copy, scale=rec[:bn],
                        )
                        mn = px.tile([P, 256], dt.float32, tag="mn", name="mn")
                        nc.vector.tensor_scalar(
                            out=mn[:bn], in0=xv[:bn], scalar1=0.0,
                            scalar2=None, op0=ALU.min,
                        )
                        ee = px.tile([P, 256], dt.float32, tag="ee", name="ee")
                        nc.scalar.activation(ee[:bn], mn[:bn], ACT.Exp)
                        mx1 = px.tile([P, 256], dt.float32, tag="mx1", name="mx1")
                        nc.vector.tensor_scalar(
                            out=mx1[:bn], in0=xv[:bn], scalar1=0.0,
                            scalar2=-1.0, op0=ALU.max, op1=ALU.add,
                        )
                        nc.vector.tensor_tensor(
                            out=a16[:bn], in0=ee[:bn], in1=mx1[:bn], op=ALU.add,
                        )
                    npar = (l + 1) % 2
                    for k in range(KCH):
                        ps = psum_tr.tile([P, P], dt.float16)
                        nc.tensor.transpose(
                            out=ps[:P, :bn],
                            in_=a16[:bn, k * P : (k + 1) * P],
                            identity=id16[:bn, :bn],
                        )
                        nc.vector.tensor_copy(
                            out=xT[npar][k][bb][:, :bn], in_=ps[:, :bn],
                        )
                    z_phase_block(l + 1, bb, bn)
                if not last:
                    allgather(l + 1)

    nc.compile()
    return nc


# ---------------------------------------------------------------------------
# entry point
# ---------------------------------------------------------------------------

_CACHE = {}


def _prepare(src, dst, n_nodes):
    key = (int(n_nodes), src.tobytes(), dst.tobytes())
    kh = hash(key)
    if kh not in _CACHE:
        meta, per_core = build_plan(src, dst, n_nodes, N_CORES)
        nc = build_nc(meta, (DIM_IN, DIM_HID, DIM_OUT))
        _CACHE[kh] = (meta, per_core, nc)
    return _CACHE[kh]


def kernel(h, src, dst, n_nodes, W1, A1, W2, A2, W3, A3):
    from concourse.bass_utils import run_bass_kernel_spmd

    n_nodes = int(n_nodes)
    assert n_nodes == N_NODES
    meta, per_core, nc = _prepare(np.asarray(src), np.asarray(dst), n_nodes)
    npc = meta["npc"]

    w = [build_waug(W1, A1), build_waug(W2, A2), build_waug(W3, A3)]
    h = np.asarray(h, dtype=np.float32)

    in_maps = []
    for c in range(N_CORES):
        in_maps.append(
            {
                "h": np.ascontiguousarray(h[c * npc : (c + 1) * npc]),
                "w0": w[0],
                "w1": w[1],
                "w2": w[2],
                "idxlo": per_core["idxlo"][c],
                "idxhi": per_core["idxhi"][c],
                "seg": per_core["seg"][c],
                "cnts": per_core["cnts"][c],
                **const_inputs(),
            }
        )
    res = run_bass_kernel_spmd(nc, in_maps, core_ids=list(range(N_CORES)))
    out = np.concatenate([res.results[c]["out"] for c in range(N_CORES)], axis=0)
    return out[:n_nodes].astype(np.float32)


# revision 19
# speedup vs baseline: 1.1470x; 1.1470x over previous
"""3-layer GAT on Trainium2, 8-core SPMD Bass kernel (v2).

Strategy (dst-partitioned, edge-gather based):
  - Nodes partitioned contiguously across 8 cores (6250/core). Each core owns
    all edges whose dst lands in its range; segment-softmax and the weighted
    scatter-sum are core-local.
  - Per layer: per 128-node block, z_aug = x @ [W.T | W.T a_src | W.T a_dst]
    computed on PE, fp16 table rows [z | 1 | 0 | s(f32) | t(f32)] written to
    DRAM, AllGather replicates the node table to every core.
  - Edge phase, per 128-dst block: one dma_gather per (block, src-half) pulls
    z_aug rows of the block's edge sources (trailing -1 indices skip padding
    descriptors; per-core valid counts come from an SBUF table via
    value_load). Attention: e = s[src] + t[dst] (t via one-hot mask reduce),
    leaky_relu, exp (shift-free: f32 exp exact for softmax). Per-dst sums via
    PE matmul U = PT_ex.T @ Zg with a constant-1 column giving the
    denominator. Scalar engine handles exp/reciprocal/normalize/tanh.
  - The next layer's z-phase for block b is emitted right after block b's
    edge compute so layers overlap; AllGather is the only barrier.
"""

import os
import sys

import numpy as np

sys.path.insert(0, "/opt/trn_rl_repo")

import ml_dtypes  # noqa: E402

PAD_FULL = bool(int(os.environ.get("GAT_PAD_FULL", "1")))

# --- problem constants (hardcoded per contest rules) ---
N_NODES = 50000
N_EDGES = 800000
DIM_IN = 256
DIM_HID = 256
DIM_OUT = 128
N_CORES = 8

BLOCK = 128  # dst nodes per block (one U psum, seg ids 0..127)
MAXCK = 8    # max chunks (x128 descriptors) per dma_gather call
SCRATCH = 1 << 15  # SWDGE ring (SBUF bytes/partition): 2048 descriptors,
                   # so one call drains while the next one is prepared
P = 128

NEG_SLOPE = 0.01


def _cdiv(a, b):
    return -(-a // b)


# ---------------------------------------------------------------------------
# host-side graph plan (pure index preprocessing)
# ---------------------------------------------------------------------------

def build_plan(src, dst, n_nodes, n_cores):
    src = np.asarray(src).astype(np.int64)
    dst = np.asarray(dst).astype(np.int64)
    assert n_nodes % n_cores == 0
    npc = n_nodes // n_cores
    H = (n_nodes + 1) // 2
    assert max(H, n_nodes - H) <= 32767

    n_blocks = _cdiv(npc, BLOCK)
    c = dst // npc
    loc = dst - c * npc
    b = loc // BLOCK
    sg = loc % BLOCK
    hf = (src >= H).astype(np.int64)

    key = (c * n_blocks + b) * 2 + hf
    cnt = np.bincount(key, minlength=n_cores * n_blocks * 2).reshape(
        n_cores, n_blocks, 2
    )
    mx = cnt.max(axis=0)  # [n_blocks, 2]
    ck = (mx + P - 1) // P  # chunks per (block, half), shared across cores
    dead = ck.sum(axis=1) == 0
    ck[dead, 0] = 1  # keep >=1 chunk per block so PSUM init happens

    order = np.lexsort((src, hf, b, c))
    gsize = cnt.reshape(-1)
    gstart = np.zeros_like(gsize)
    gstart[1:] = np.cumsum(gsize)[:-1]

    # block/call layout (shared across cores)
    blocks = []
    seg_cols = lo_cols = hi_cols = 0
    ncalls = 0
    for bb in range(n_blocks):
        cklo, ckhi = int(ck[bb, 0]), int(ck[bb, 1])
        calls = []
        for h, ckh, off8 in ((0, cklo, lo_cols), (1, ckhi, hi_cols)):
            dbase = 0 if h == 0 else cklo
            for c0 in range(0, ckh, MAXCK):
                nck = min(MAXCK, ckh - c0)
                calls.append(
                    dict(h=h, c0=dbase + c0, u0=c0, nck=nck,
                         ci=ncalls, off8=off8 + c0 * 8)
                )
                ncalls += 1
        blocks.append(
            dict(cs=seg_cols, cklo=cklo, ckhi=ckhi, ctot=cklo + ckhi,
                 lo8=lo_cols, hi8=hi_cols, calls=calls)
        )
        seg_cols += cklo + ckhi
        lo_cols += cklo * 8
        hi_cols += ckhi * 8

    ncalls_pad = _cdiv(ncalls, 16) * 16

    def wrap16(a):
        # idx j -> partition j%16, col j//16; replicated to 8 groups of 16
        S = len(a) // 16
        w = a.reshape(S, 16).T
        return np.tile(w, (8, 1))

    idxlo = np.full((n_cores, P, lo_cols), -1, np.int16)
    idxhi = np.full((n_cores, P, hi_cols), -1, np.int16)
    seg = np.full((n_cores, P, seg_cols), 255.0, np.float32)
    cnts = np.zeros((n_cores, 1, ncalls_pad), np.int32)

    for cc_ in range(n_cores):
        for bb, blk in enumerate(blocks):
            ctot = blk["ctot"]
            seg_slots = np.full(ctot * P, 255, np.int64)
            for h, ckh, off8, idxarr, sbase in (
                (0, blk["cklo"], blk["lo8"], idxlo, 0),
                (1, blk["ckhi"], blk["hi8"], idxhi, blk["cklo"] * P),
            ):
                if ckh == 0:
                    continue
                n = cnt[cc_, bb, h]
                s0 = gstart[(cc_ * n_blocks + bb) * 2 + h]
                e = order[s0 : s0 + n]
                vals = np.full(ckh * P, -1, np.int64)
                vals[:n] = src[e] - (H if h else 0)
                seg_slots[sbase : sbase + n] = sg[e]
                # per-call dummy fill + counts
                for call in blk["calls"]:
                    if call["h"] != h:
                        continue
                    lo_s = call["u0"] * P
                    cap = call["nck"] * P
                    v = min(max(n - lo_s, 0), cap)
                    if v == 0:
                        vals[lo_s] = 0  # dummy valid idx (seg stays 255)
                        v = 1
                    # the SWDGE ring booking must match the descriptors the
                    # ucode actually pushes: valid-prefix count with trailing
                    # -1 trim, full capacity when padded with dummy indices
                    cnts[cc_, 0, call["ci"]] = cap if PAD_FULL else v
                if PAD_FULL:
                    vals[vals < 0] = 0
                idxarr[cc_, :, off8 : off8 + ckh * 8] = wrap16(vals)
            seg[cc_, :, blk["cs"] : blk["cs"] + ctot] = seg_slots.reshape(
                ctot, P
            ).T

    meta = dict(
        n_cores=n_cores,
        n_nodes=n_nodes,
        npc=npc,
        H=H,
        n_blocks=n_blocks,
        blocks=blocks,
        lo_cols=lo_cols,
        hi_cols=hi_cols,
        seg_cols=seg_cols,
        ncalls=ncalls,
        ncalls_pad=ncalls_pad,
        ctot_max=max(bl["ctot"] for bl in blocks),
    )
    per_core = dict(
        idxlo=idxlo,
        idxhi=idxhi,
        seg=seg.astype(ml_dtypes.bfloat16),
        cnts=cnts,
    )
    return meta, per_core


def const_inputs():
    iota = np.tile(np.arange(BLOCK, dtype=np.float32), (P, 1)).astype(
        ml_dtypes.bfloat16
    )
    id16 = np.eye(P, dtype=np.float16)
    id32 = np.eye(P, dtype=np.float32)
    return {"iota": iota, "id16": id16, "id32": id32}


def build_waug(W, A):
    d_out = W.shape[0]
    Wt = W.T.astype(np.float64)
    a_s = A[0, :d_out].astype(np.float64)
    a_d = A[0, d_out:].astype(np.float64)
    waug = np.concatenate([Wt, (Wt @ a_s)[:, None], (Wt @ a_d)[:, None]], axis=1)
    return waug.astype(np.float16)


# ---------------------------------------------------------------------------
# device program
# ---------------------------------------------------------------------------

def build_nc(meta, dims, debug=False, timing_single_core=False,
             no_collective=False):
    import concourse.bacc as bacc
    import concourse.bass as bass
    import concourse.mybir as mybir
    import concourse.tile as tile
    from concourse.library_config import mlp

    dt = mybir.dt
    AP = bass.AP
    ALU = bass.mybir.AluOpType
    ACT = bass.mybir.ActivationFunctionType
    d_in, d_hid, d_out = dims
    npc = meta["npc"]
    H = meta["H"]
    N = meta["n_nodes"]
    n_blocks = meta["n_blocks"]
    n_cores = meta["n_cores"]
    blocks = meta["blocks"]
    ctmax = meta["ctot_max"]
    KCH = d_in // P  # contraction chunks (2)
    assert d_in == d_hid == 2 * P and d_out == P

    #        (Din,  Dout,  act,    stride)
    LYR = [
        (d_in, d_hid, "tanh", 384),
        (d_hid, d_hid, "elu", 384),
        (d_hid, d_out, None, 256),
    ]

    nc = bacc.Bacc(
        "TRN2", target_bir_lowering=False, debug=debug,
        num_devices=1 if timing_single_core else n_cores,
        dynamic_dma_scratch_size=SCRATCH,
    )

    h_in = nc.dram_tensor("h", [npc, d_in], dt.float32, kind="ExternalInput")
    w_in = [
        nc.dram_tensor(f"w{l}", [LYR[l][0], LYR[l][1] + 2], dt.float16,
                       kind="ExternalInput")
        for l in range(3)
    ]
    ixlo_in = nc.dram_tensor("idxlo", [P, meta["lo_cols"]], dt.int16,
                             kind="ExternalInput")
    ixhi_in = nc.dram_tensor("idxhi", [P, meta["hi_cols"]], dt.int16,
                             kind="ExternalInput")
    seg_in = nc.dram_tensor("seg", [P, meta["seg_cols"]], dt.bfloat16,
                            kind="ExternalInput")
    cnts_in = nc.dram_tensor("cnts", [1, meta["ncalls_pad"]], dt.int32,
                             kind="ExternalInput")
    iota_in = nc.dram_tensor("iota", [P, BLOCK], dt.bfloat16, kind="ExternalInput")
    id16_in = nc.dram_tensor("id16", [P, P], dt.float16, kind="ExternalInput")
    id32_in = nc.dram_tensor("id32", [P, P], dt.float32, kind="ExternalInput")
    out_t = nc.dram_tensor("out", [npc, d_out], dt.float32, kind="ExternalOutput")

    agi = [nc.dram_tensor(f"agi{l}", [npc, LYR[l][3]], dt.float16) for l in range(3)]
    ago = [
        nc.dram_tensor(f"ago{l}", [npc * n_cores, LYR[l][3]], dt.float16,
                       addr_space="Shared")
        for l in range(3)
    ]

    def bc_mid(ap2, n):
        # [P, W] -> [P, n, W] broadcasting a middle dim
        return AP(ap2.tensor, ap2.offset, [ap2.ap[0], [0, n], ap2.ap[1]])

    def bc_last(ap2, n):
        # [P, W] -> [P, W, n] broadcasting the last dim
        return AP(ap2.tensor, ap2.offset, [ap2.ap[0], ap2.ap[1], [0, n]])

    with tile.TileContext(nc) as tc:
        import contextlib

        ctx = contextlib.ExitStack()
        with ctx:
            pers = ctx.enter_context(tc.tile_pool(name="pers", bufs=1))
            pg = ctx.enter_context(tc.tile_pool(name="pg", bufs=3))
            ppt = ctx.enter_context(tc.tile_pool(name="ppt", bufs=2))
            psm = ctx.enter_context(tc.tile_pool(name="psm", bufs=4))
            pz = ctx.enter_context(tc.tile_pool(name="pz", bufs=3))
            px = ctx.enter_context(tc.tile_pool(name="px", bufs=2))
            psum_z = ctx.enter_context(tc.tile_pool(name="psz", bufs=2, space="PSUM"))
            psum_u = ctx.enter_context(tc.tile_pool(name="psu", bufs=2, space="PSUM"))
            psum_tr = ctx.enter_context(tc.tile_pool(name="pstr", bufs=2, space="PSUM"))
            psum_tt = ctx.enter_context(tc.tile_pool(name="pstt", bufs=2, space="PSUM"))

            nc.gpsimd.load_library(mlp)

            # persistent state
            seg_sb = pers.tile([P, meta["seg_cols"]], dt.bfloat16, tag="seg", name="seg_sb")
            ixlo_sb = pers.tile([P, meta["lo_cols"]], dt.int16, tag="ixlo", name="ixlo_sb")
            ixhi_sb = pers.tile([P, meta["hi_cols"]], dt.int16, tag="ixhi", name="ixhi_sb")
            cnt_sb = pers.tile([1, meta["ncalls_pad"]], dt.int32, tag="cnts", name="cnt_sb")
            W_sb = [pers.tile([P, KCH, LYR[l][1] + 2], dt.float16, tag=f"w{l}", name=f"wsb{l}")
                    for l in range(3)]
            iota_sb = pers.tile([P, BLOCK], dt.bfloat16, tag="iota", name="iota_sb")
            id16 = pers.tile([P, P], dt.float16, tag="id16", name="id16")
            id32 = pers.tile([P, P], dt.float32, tag="id32", name="id32")
            t_row = [
                pers.tile([1, n_blocks * BLOCK], dt.bfloat16, tag=f"trow{par}",
                          name=f"t_row{par}")
                for par in range(2)
            ]
            xT = [
                [
                    [pers.tile([P, BLOCK], dt.float16, tag=f"xT{par}_{k}_{bb}",
                               name=f"xT{par}_{k}_{bb}") for bb in range(n_blocks)]
                    for k in range(KCH)
                ]
                for par in range(2)
            ]

            nc.sync.dma_start(out=seg_sb[:], in_=seg_in[:, :])
            nc.sync.dma_start(out=ixlo_sb[:], in_=ixlo_in[:, :])
            nc.sync.dma_start(out=ixhi_sb[:], in_=ixhi_in[:, :])
            nc.sync.dma_start(out=cnt_sb[:], in_=cnts_in[:, :])
            for l in range(3):
                nc.sync.dma_start(
                    out=W_sb[l][:],
                    in_=w_in[l].ap().rearrange("(k p) d -> p k d", p=P),
                )
            nc.sync.dma_start(out=iota_sb[:], in_=iota_in[:, :])
            nc.sync.dma_start(out=id16[:], in_=id16_in[:, :])
            nc.sync.dma_start(out=id32[:], in_=id32_in[:, :])
            nc.vector.memset(t_row[0][:], 0.0)
            nc.vector.memset(t_row[1][:], 0.0)

            def z_phase_block(l, bb, bn):
                """z_aug for nodes of block bb of layer l -> agi[l] rows."""
                Dout = LYR[l][1]
                DU = Dout + 2
                ASM = Dout + 6
                par = l % 2
                zp = psum_z.tile([P, 258], dt.float32, tag="zp", name="zp")
                for k in range(KCH):
                    nc.tensor.matmul(
                        out=zp[:bn, :DU],
                        lhsT=xT[par][k][bb][:, :bn],
                        rhs=W_sb[l][:, k, :DU],
                        start=(k == 0),
                        stop=(k == KCH - 1),
                    )
                asm = pz.tile([P, 262], dt.float16, tag="asm", name="asm")
                nc.scalar.activation(asm[:bn, 0:Dout], zp[:bn, 0:Dout], ACT.Copy)
                nc.vector.memset(asm[:bn, Dout : Dout + 1], 1.0)
                nc.vector.memset(asm[:bn, Dout + 1 : Dout + 2], 0.0)
                nc.vector.tensor_copy(
                    out=asm[:bn, Dout + 2 : Dout + 6].bitcast(dt.float32),
                    in_=zp[:bn, Dout : Dout + 2],
                )
                tcol = pz.tile([P, 1], dt.float32, tag="tcol", name="tcol")
                nc.vector.tensor_copy(out=tcol[:bn], in_=zp[:bn, Dout + 1 : Dout + 2])
                tp = psum_tt.tile([1, P], dt.float32)
                nc.tensor.transpose(
                    out=tp[:1, :bn], in_=tcol[:bn, :], identity=id32[:bn, :bn],
                )
                nc.vector.tensor_copy(
                    out=t_row[par][0:1, bb * BLOCK : bb * BLOCK + bn],
                    in_=tp[0:1, :bn],
                )
                nc.sync.dma_start(
                    out=agi[l][bb * BLOCK : bb * BLOCK + bn, 0:ASM],
                    in_=asm[:bn, 0:ASM],
                )

            def allgather(l):
                if timing_single_core or no_collective:
                    nc.sync.dma_start(out=ago[l].ap()[0:npc, :], in_=agi[l].ap()[:, :])
                else:
                    nc.gpsimd.collective_compute(
                        "AllGather",
                        ALU.bypass,
                        replica_groups=[list(range(n_cores))],
                        ins=[agi[l].ap().opt()],
                        outs=[ago[l].ap().opt()],
                    )

            # ---- layer-0 input: load h, cast fp16, transpose to xT[0] ----
            for bb in range(n_blocks):
                bn = min(BLOCK, npc - bb * BLOCK)
                ht = pz.tile([P, d_in], dt.float32, tag="ht", name="ht")
                nc.sync.dma_start(out=ht[:bn], in_=h_in[bb * BLOCK : bb * BLOCK + bn, :])
                h16 = pz.tile([P, d_in], dt.float16, tag="h16", name="h16")
                nc.scalar.activation(h16[:bn], ht[:bn], ACT.Copy)
                for k in range(KCH):
                    ps = psum_tr.tile([P, P], dt.float16)
                    nc.tensor.transpose(
                        out=ps[:P, :bn],
                        in_=h16[:bn, k * P : (k + 1) * P],
                        identity=id16[:bn, :bn],
                    )
                    nc.vector.tensor_copy(out=xT[0][k][bb][:, :bn], in_=ps[:, :bn])
                z_phase_block(0, bb, bn)
            allgather(0)

            # ---- layers ----
            qn = [0]  # gather call counter

            for l in range(3):
                Din, Dout, act, STRIDE = LYR[l]
                ELEM = STRIDE
                DU = Dout + 2
                SOFF = Dout + 2
                par = l % 2
                last = l == 2

                lo_tab = ago[l].ap()[0:H, 0:ELEM]
                hi_tab = ago[l].ap()[H:N, 0:ELEM]

                # re-zero the g ring buffers: stale bytes from the previous
                # layer's layout could bitcast to inf/nan in pad slots
                for _ in range(3):
                    gg = pg.tile([P, ctmax, ELEM], dt.float16, tag="g", name="g")
                    nc.vector.memset(gg[:], 0.0)

                for bb, blk in enumerate(blocks):
                    bn = min(BLOCK, npc - bb * BLOCK)
                    cs = blk["cs"]
                    ctot = blk["ctot"]
                    g = pg.tile([P, ctmax, ELEM], dt.float16, tag="g", name="g")
                    for call in blk["calls"]:
                        tabsrc = hi_tab if call["h"] else lo_tab
                        ixsb = ixhi_sb if call["h"] else ixlo_sb
                        nck = call["nck"]
                        # PAD_FULL: all index slots are valid (dummy idx 0 for
                        # padding), so the static count matches the SWDGE ring
                        # booking on every core
                        nc.gpsimd.dma_gather(
                            g[:, call["c0"] : call["c0"] + nck, :],
                            tabsrc,
                            ixsb[:, call["off8"] : call["off8"] + nck * 8],
                            nck * P,
                            nck * P,
                            ELEM,
                            elem_step=STRIDE,
                        )
                        qn[0] += 1

                    # t broadcast for this block's dst nodes
                    tb = psm.tile([P, BLOCK], dt.bfloat16, tag="tb", name="tb")
                    nc.gpsimd.partition_broadcast(
                        tb[:],
                        t_row[par][0:1, bb * BLOCK : bb * BLOCK + BLOCK],
                        channels=P,
                    )
                    seg_v = seg_sb[:, cs : cs + ctot]
                    pt0 = ppt.tile([P, ctmax, BLOCK], dt.bfloat16, tag="pt0", name="pt0")
                    nc.vector.tensor_tensor(
                        out=pt0[:, :ctot, :],
                        in0=bc_last(seg_v, BLOCK),
                        in1=bc_mid(iota_sb[:], ctot),
                        op=ALU.is_equal,
                    )
                    ptm = ppt.tile([P, ctmax, BLOCK], dt.bfloat16, tag="ptm", name="ptm")
                    nc.vector.tensor_tensor(
                        out=ptm[:, :ctot, :],
                        in0=pt0[:, :ctot, :],
                        in1=bc_mid(tb[:], ctot),
                        op=ALU.mult,
                    )
                    tsel = psm.tile([P, ctmax], dt.float32, tag="tsel", name="tsel")
                    nc.vector.tensor_reduce(
                        out=tsel[:, :ctot],
                        in_=ptm[:, :ctot, :],
                        axis=bass.mybir.AxisListType.X,
                        op=ALU.add,
                    )
                    # e = s + t  (s: f32 bits at fp16 cols SOFF..SOFF+2)
                    sv = g[:, 0:ctot, SOFF : SOFF + 2].bitcast(dt.float32)
                    sv2 = AP(sv.tensor, sv.offset, [sv.ap[0], sv.ap[1]])
                    e0 = psm.tile([P, ctmax], dt.float32, tag="e0", name="e0")
                    nc.vector.tensor_tensor(
                        out=e0[:, :ctot], in0=sv2, in1=tsel[:, :ctot], op=ALU.add,
                    )
                    e1 = psm.tile([P, ctmax], dt.float32, tag="e1", name="e1")
                    nc.vector.tensor_scalar(
                        out=e1[:, :ctot], in0=e0[:, :ctot], scalar1=NEG_SLOPE,
                        scalar2=None, op0=ALU.mult,
                    )
                    e2 = psm.tile([P, ctmax], dt.float32, tag="e2", name="e2")
                    nc.vector.tensor_tensor(
                        out=e2[:, :ctot], in0=e0[:, :ctot], in1=e1[:, :ctot],
                        op=ALU.max,
                    )
                    exb = psm.tile([P, ctmax], dt.bfloat16, tag="exb", name="exb")
                    nc.scalar.activation(exb[:, :ctot], e2[:, :ctot], ACT.Exp)
                    ptx = ppt.tile([P, ctmax, BLOCK], dt.bfloat16, tag="ptx", name="ptx")
                    nc.vector.tensor_tensor(
                        out=ptx[:, :ctot, :],
                        in0=pt0[:, :ctot, :],
                        in1=bc_last(exb[:, :ctot], BLOCK),
                        op=ALU.mult,
                    )
                    U = psum_u.tile([P, 258], dt.float32, tag="U", name="U")
                    for k in range(ctot):
                        nc.tensor.matmul(
                            out=U[:, :DU],
                            lhsT=ptx[:, k, :],
                            rhs=g[:, k, 0:DU],
                            start=(k == 0),
                            stop=(k == ctot - 1),
                        )
                    den = psm.tile([P, 1], dt.float32, tag="den", name="den")
                    nc.vector.tensor_scalar(
                        out=den[:bn],
                        in0=U[:bn, Dout : Dout + 1],
                        scalar1=1e-9, scalar2=None, op0=ALU.max,
                    )
                    rec = psm.tile([P, 1], dt.float32, tag="rec", name="rec")
                    nc.vector.reciprocal(rec[:bn], den[:bn])
                    if last:
                        ox = px.tile([P, d_out], dt.float32, tag="ox", name="ox")
                        nc.scalar.activation(
                            ox[:bn], U[:bn, 0:d_out], ACT.Copy, scale=rec[:bn],
                        )
                        nc.sync.dma_start(
                            out=out_t[bb * BLOCK : bb * BLOCK + bn, :],
                            in_=ox[:bn, :],
                        )
                        continue
                    # activation + transpose into next layer's xT
                    a16 = px.tile([P, 256], dt.float16, tag="a16", name="a16")
                    if act == "tanh":
                        nc.scalar.activation(
                            a16[:bn], U[:bn, 0:Dout], ACT.Tanh, scale=rec[:bn],
                        )
                    else:  # elu = max(x,0) + exp(min(x,0)) - 1
                        xv = px.tile([P, 256], dt.float32, tag="xv", name="xv")
                        nc.scalar.activation(
                            xv[:bn], U[:bn, 0:Dout], ACT.Copy, scale=rec[:bn],
                        )
                        mn = px.tile([P, 256], dt.float32, tag="mn", name="mn")
                        nc.vector.tensor_scalar(
                            out=mn[:bn], in0=xv[:bn], scalar1=0.0,
                            scalar2=None, op0=ALU.min,
                        )
                        ee = px.tile([P, 256], dt.float32, tag="ee", name="ee")
                        nc.scalar.activation(ee[:bn], mn[:bn], ACT.Exp)
                        mx1 = px.tile([P, 256], dt.float32, tag="mx1", name="mx1")
                        nc.vector.tensor_scalar(
                            out=mx1[:bn], in0=xv[:bn], scalar1=0.0,
                            scalar2=-1.0, op0=ALU.max, op1=ALU.add,
                        )
                        nc.vector.tensor_tensor(
                            out=a16[:bn], in0=ee[:bn], in1=mx1[:bn], op=ALU.add,
                        )
                    npar = (l + 1) % 2
                    for k in range(KCH):
                        ps = psum_tr.tile([P, P], dt.float16)
                        nc.tensor.transpose(
                            out=ps[:P, :bn],
                            in_=a16[:bn, k * P : (k + 1) * P],
                            identity=id16[:bn, :bn],
                        )
                        nc.vector.tensor_copy(
                            out=xT[npar][k][bb][:, :bn], in_=ps[:, :bn],
                        )
                    z_phase_block(l + 1, bb, bn)
                if not last:
                    allgather(l + 1)

    nc.compile()
    return nc


# ---------------------------------------------------------------------------
# entry point
# ---------------------------------------------------------------------------

_CACHE = {}


def _prepare(src, dst, n_nodes):
    key = (int(n_nodes), src.tobytes(), dst.tobytes())
    kh = hash(key)
    if kh not in _CACHE:
        meta, per_core = build_plan(src, dst, n_nodes, N_CORES)
        nc = build_nc(meta, (DIM_IN, DIM_HID, DIM_OUT))
        _CACHE[kh] = (meta, per_core, nc)
    return _CACHE[kh]


def kernel(h, src, dst, n_nodes, W1, A1, W2, A2, W3, A3):
    from concourse.bass_utils import run_bass_kernel_spmd

    n_nodes = int(n_nodes)
    assert n_nodes == N_NODES
    meta, per_core, nc = _prepare(np.asarray(src), np.asarray(dst), n_nodes)
    npc = meta["npc"]

    w = [build_waug(W1, A1), build_waug(W2, A2), build_waug(W3, A3)]
    h = np.asarray(h, dtype=np.float32)

    in_maps = []
    for c in range(N_CORES):
        in_maps.append(
            {
                "h": np.ascontiguousarray(h[c * npc : (c + 1) * npc]),
                "w0": w[0],
                "w1": w[1],
                "w2": w[2],
                "idxlo": per_core["idxlo"][c],
                "idxhi": per_core["idxhi"][c],
                "seg": per_core["seg"][c],
                "cnts": per_core["cnts"][c],
                **const_inputs(),
            }
        )
    res = run_bass_kernel_spmd(nc, in_maps, core_ids=list(range(N_CORES)))
    out = np.concatenate([res.results[c]["out"] for c in range(N_CORES)], axis=0)
    return out[:n_nodes].astype(np.float32)
